# revision 5
# baseline (speedup 1.0000x reference)
"""GCN + DiffPool kernel for Trainium2, data-parallel over graphs across 8 NeuronCores.

Model (per graph, n=150 nodes):
  Z1 = relu(An @ (x @ W1) + b1)          An = D^-1/2 (A+I) D^-1/2
  Z2 = relu(An @ (Z1 @ W2) + b2)
  S  = softmax(An @ (Z2 @ Wa) + ba)      [n, 25]
  Zp = S^T @ Z2 ; Ap = S^T @ (A @ S)
  H  = relu(Anp @ (Zp @ Wp) + bp)        pooled GCN, 25 cluster-nodes
  logits = (sum_rows H) @ Wc + bc

Sharding: 64 graphs -> 8 devices x 8 graphs. The batch adjacency is block
diagonal, so each device only receives its 8 graphs' 150x150 diagonal blocks
(packed into a [128,8,150] + [22,8,150] partition-chunk layout) and its node
rows of x (shipped feature-major). Everything is graph-local; the final [8,10]
logits per device are concatenated on host.

On-device layout convention:
  fm (feature-major): [feat_part, graph, node]  - used for W-multiplies (lhsT)
  nm (node-major):    [node_part, graph, feat]  - used for A-multiplies
A-multiplies contract over nodes, so node dim (150) is split into partition
chunks c0=[0:128], c1=[128:150]. Normalization is folded: the row factor
D^-1/2 is applied to the moving operand; the column factor is materialized
once as An_col = (A+I) * dT_broadcast.
"""

import numpy as np

import concourse.bass as bass
import concourse.mybir as mybir
import concourse.tile as tile
from concourse import bacc
from concourse.bass_utils import run_bass_kernel_spmd

F32 = mybir.dt.float32
AF = mybir.ActivationFunctionType

N_NODES = 9600
N_FEAT = 128
HIDDEN = 64
CLUSTERS = 25
NUM_CLASSES = 10
B_GRAPHS = 64
NPG = 150            # nodes per graph
DEV = 8              # devices
GPD = 8              # graphs per device
C0, C1 = 128, 22     # node partition chunks (128 + 22 = 150)

_CACHE = {}


def _chunk(c):
    """(offset, size) of node chunk c."""
    return (0, C0) if c == 0 else (C0, C1)


def build_nc():
    nc = bacc.Bacc("TRN2", target_bir_lowering=False, debug=False, num_devices=DEV)

    def din(name, shape):
        return nc.dram_tensor(name, shape, F32, kind="ExternalInput").ap()

    xT = din("xT", [N_FEAT, GPD, NPG])
    a0 = din("a0", [C0, GPD, NPG])
    a1 = din("a1", [C1, GPD, NPG])
    id0 = din("id0", [C0, NPG])
    id1 = din("id1", [C1, NPG])
    id25 = din("id25", [CLUSTERS, CLUSTERS])
    id64 = din("id64", [HIDDEN, HIDDEN])
    W1 = din("W1", [N_FEAT, HIDDEN])
    W2 = din("W2", [HIDDEN, HIDDEN])
    Wa = din("Wa", [HIDDEN, CLUSTERS])
    Wp = din("Wp", [HIDDEN, HIDDEN])
    Wc = din("Wc", [HIDDEN, NUM_CLASSES])
    b1 = din("b1", [HIDDEN, 1])
    b2 = din("b2", [HIDDEN, 1])
    bp = din("bp", [HIDDEN, 1])
    ba = din("ba", [1, CLUSTERS])
    bc = din("bc", [1, NUM_CLASSES])
    out = nc.dram_tensor("out", [GPD, NUM_CLASSES], F32, kind="ExternalOutput").ap()

    with tile.TileContext(nc) as tc:
        with (
            tc.tile_pool(name="cst", bufs=1) as cst,
            tc.tile_pool(name="act", bufs=1) as act,
            tc.tile_pool(name="ps", bufs=8, space="PSUM") as ps,
            tc.tile_pool(name="dram", bufs=1, space="DRAM") as dram,
        ):
            # ---- load inputs -------------------------------------------------
            def load(dst, src):
                nc.gpsimd.dma_start(out=dst, in_=src)

            s_xT = cst.tile([N_FEAT, GPD, NPG], F32, tag="xT")
            load(s_xT[:], xT)
            s_a0 = cst.tile([C0, GPD, NPG], F32, tag="a0")
            load(s_a0[:], a0)
            s_a1 = cst.tile([C1, GPD, NPG], F32, tag="a1")
            load(s_a1[:], a1)
            s_id0 = cst.tile([C0, NPG], F32, tag="id0")
            load(s_id0[:], id0)
            s_id1 = cst.tile([C1, NPG], F32, tag="id1")
            load(s_id1[:], id1)
            s_id25 = cst.tile([CLUSTERS, CLUSTERS], F32, tag="id25")
            load(s_id25[:], id25)
            s_id64 = cst.tile([HIDDEN, HIDDEN], F32, tag="id64")
            load(s_id64[:], id64)
            s_W1 = cst.tile([N_FEAT, HIDDEN], F32, tag="W1")
            load(s_W1[:], W1)
            s_W2 = cst.tile([HIDDEN, HIDDEN], F32, tag="W2")
            load(s_W2[:], W2)
            s_Wa = cst.tile([HIDDEN, CLUSTERS], F32, tag="Wa")
            load(s_Wa[:], Wa)
            s_Wp = cst.tile([HIDDEN, HIDDEN], F32, tag="Wp")
            load(s_Wp[:], Wp)
            s_Wc = cst.tile([HIDDEN, NUM_CLASSES], F32, tag="Wc")
            load(s_Wc[:], Wc)
            s_b1 = cst.tile([HIDDEN, 1], F32, tag="b1")
            load(s_b1[:], b1)
            s_b2 = cst.tile([HIDDEN, 1], F32, tag="b2")
            load(s_b2[:], b2)
            s_bp = cst.tile([HIDDEN, 1], F32, tag="bp")
            load(s_bp[:], bp)
            s_ba = cst.tile([C0, CLUSTERS], F32, tag="ba")
            load(s_ba[:], ba.broadcast_to((C0, CLUSTERS)))
            s_bc = cst.tile([GPD, NUM_CLASSES], F32, tag="bc")
            load(s_bc[:], bc.broadcast_to((GPD, NUM_CLASSES)))

            s_a = (s_a0, s_a1)
            s_id = (s_id0, s_id1)

            # ---- degrees + d = rsqrt(deg+1) ---------------------------------
            # deg excludes the self loop; the +1 in the bias accounts for it.
            s_d = []
            for c, cn in ((0, C0), (1, C1)):
                deg = act.tile([cn, GPD], F32, tag=f"deg{c}")
                nc.vector.reduce_sum(out=deg[:], in_=s_a[c][:], axis=mybir.AxisListType.X)
                nc.vector.tensor_scalar_add(deg[:], deg[:], 1.0)
                rec = act.tile([cn, GPD], F32, tag=f"rec{c}")
                nc.vector.reciprocal(rec[:], deg[:])
                d = act.tile([cn, GPD], F32, tag=f"d{c}")
                nc.scalar.sqrt(d[:], rec[:])
                s_d.append(d)

            # ---- dT broadcast via DRAM roundtrip ----------------------------
            # d is [node_chunk, graph]; we need dT_bc[p, g, j] = d_g[j] on all
            # partitions. Scatter to DRAM in (g, j) layout, read back with a
            # partition-broadcast AP.
            dTd = dram.tile([GPD * NPG], F32, tag="dTd")
            dT_scat = dTd[:].rearrange("(g j) -> j g", g=GPD)
            nc.gpsimd.dma_start(out=dT_scat[0:C0, :], in_=s_d[0][:])
            nc.gpsimd.dma_start(out=dT_scat[C0:NPG, :], in_=s_d[1][:])
            s_dT = cst.tile([C0, GPD, NPG], F32, tag="dT")
            dT_src = dTd[:].rearrange("(g j) -> g j", g=GPD)[None, :, :]
            nc.gpsimd.dma_start(out=s_dT[:], in_=dT_src.broadcast_to((C0, GPD, NPG)))

            # ---- An_col = (A + I) * dT_bc -----------------------------------
            s_An = []
            for c, cn in ((0, C0), (1, C1)):
                ah = act.tile([cn, GPD, NPG], F32, tag=f"ah{c}")
                idb = s_id[c][:][:, None, :].broadcast_to((cn, GPD, NPG))
                nc.gpsimd.tensor_add(ah[:], s_a[c][:], idb)
                an = act.tile([cn, GPD, NPG], F32, tag=f"an{c}")
                nc.vector.tensor_mul(an[:], ah[:], s_dT[0:cn, :, :])
                s_An.append(an)

            # ---- helpers ----------------------------------------------------
            def w_mult_nm(lhs_fm, w, kdim, fout, name, scale=True):
                """nm out: lhsT = fm activation slice [kdim, node_chunk],
                rhs = w [kdim, fout]. Returns (nm0, nm1) scaled by d."""
                outs = []
                for c, cn in ((0, C0), (1, C1)):
                    off, _ = _chunk(c)
                    p = ps.tile([cn, GPD, fout], F32, tag="ps")
                    for g in range(GPD):
                        nc.tensor.matmul(
                            p[:, g, :], lhs_fm[0:kdim, g, off:off + cn], w[:],
                            start=True, stop=True,
                        )
                    o = act.tile([cn, GPD, fout], F32, tag=f"{name}{c}")
                    if scale:
                        dbc = s_d[c][:][:, :, None].broadcast_to((cn, GPD, fout))
                        nc.vector.tensor_mul(o[:], p[:], dbc)
                    else:
                        nc.vector.tensor_copy(o[:], p[:])
                    outs.append(o)
                return outs

            def an_mult_fm(m_nm, bias, name):
                """fm out [HIDDEN, g, NPG] = relu((An @ M) + bias) per graph.
                lhsT = M_nm chunk [k, HIDDEN], rhs = An_col chunk [k, NPG]."""
                o = act.tile([HIDDEN, GPD, NPG], F32, tag=name)
                for grp in range(3):           # graphs packed 3|3|2 per PSUM bank
                    gs = range(3 * grp, min(3 * grp + 3, GPD))
                    p = ps.tile([HIDDEN, len(gs), NPG], F32, tag="ps")
                    for i, g in enumerate(gs):
                        for c, cn in ((0, C0), (1, C1)):
                            nc.tensor.matmul(
                                p[:, i, :], m_nm[c][0:cn, g, :], s_An[c][0:cn, g, :],
                                start=(c == 0), stop=(c == 1),
                            )
                    nc.scalar.activation(o[:, list(gs)[0]:list(gs)[-1] + 1, :], p[:],
                                         AF.Relu, bias=bias[:])
                return o

            # ---- encoder ----------------------------------------------------
            m1 = w_mult_nm(s_xT, s_W1, N_FEAT, HIDDEN, "m1")
            z1 = an_mult_fm(m1, s_b1, "z1")                       # [64, g, 150] fm
            m2 = w_mult_nm(z1, s_W2, HIDDEN, HIDDEN, "m2")
            z2 = an_mult_fm(m2, s_b2, "z2")                       # [64, g, 150] fm

            # ---- Z2 transpose -> nm (for pooling contractions) --------------
            z2n = []
            for c, cn in ((0, C0), (1, C1)):
                off, _ = _chunk(c)
                p = ps.tile([cn, GPD, HIDDEN], F32, tag="ps")
                for g in range(GPD):
                    nc.tensor.transpose(p[:, g, :], z2[0:HIDDEN, g, off:off + cn],
                                        s_id64[:])
                o = act.tile([cn, GPD, HIDDEN], F32, tag=f"z2n{c}")
                nc.vector.tensor_copy(o[:], p[:])
                z2n.append(o)

            # ---- assignment: S = softmax(An @ (Z2 @ Wa) + ba), nm -----------
            v = w_mult_nm(z2, s_Wa, HIDDEN, CLUSTERS, "v")        # d-scaled
            s_S = []
            for mc, mn in ((0, C0), (1, C1)):
                moff, _ = _chunk(mc)
                p = ps.tile([mn, GPD, CLUSTERS], F32, tag="ps")
                for g in range(GPD):
                    for c, cn in ((0, C0), (1, C1)):
                        nc.tensor.matmul(
                            p[:, g, :], s_An[c][0:cn, g, moff:moff + mn],
                            v[c][0:cn, g, :], start=(c == 0), stop=(c == 1),
                        )
                sp = act.tile([mn, GPD, CLUSTERS], F32, tag=f"sp{mc}")
                bab = s_ba[:][0:mn, None, :].broadcast_to((mn, GPD, CLUSTERS))
                nc.vector.tensor_add(sp[:], p[:], bab)
                e = act.tile([mn, GPD, CLUSTERS], F32, tag=f"e{mc}")
                nc.scalar.activation(e[:], sp[:], AF.Exp)
                ssum = act.tile([mn, GPD], F32, tag=f"ssum{mc}")
                nc.vector.reduce_sum(out=ssum[:], in_=e[:], axis=mybir.AxisListType.X)
                rs = act.tile([mn, GPD], F32, tag=f"rs{mc}")
                nc.vector.reciprocal(rs[:], ssum[:])
                s = act.tile([mn, GPD, CLUSTERS], F32, tag=f"s{mc}")
                nc.vector.tensor_mul(s[:], e[:],
                                     rs[:][:, :, None].broadcast_to((mn, GPD, CLUSTERS)))
                s_S.append(s)

            # ---- AS = A @ S (raw adjacency), nm -----------------------------
            s_AS = []
            for mc, mn in ((0, C0), (1, C1)):
                moff, _ = _chunk(mc)
                p = ps.tile([mn, GPD, CLUSTERS], F32, tag="ps")
                for g in range(GPD):
                    for c, cn in ((0, C0), (1, C1)):
                        nc.tensor.matmul(
                            p[:, g, :], s_a[c][0:cn, g, moff:moff + mn],
                            s_S[c][0:cn, g, :], start=(c == 0), stop=(c == 1),
                        )
                o = act.tile([mn, GPD, CLUSTERS], F32, tag=f"as{mc}")
                nc.scalar.copy(o[:], p[:])
                s_AS.append(o)

            # ---- Zp = S^T @ Z2 (fm out), Ap = S^T @ AS (nm out) -------------
            p_zp = ps.tile([HIDDEN, GPD, CLUSTERS], F32, tag="ps")
            for g in range(GPD):
                for c, cn in ((0, C0), (1, C1)):
                    nc.tensor.matmul(p_zp[:, g, :], z2n[c][0:cn, g, :],
                                     s_S[c][0:cn, g, :], start=(c == 0), stop=(c == 1))
            s_Zp = act.tile([HIDDEN, GPD, CLUSTERS], F32, tag="zp")
            nc.vector.tensor_copy(s_Zp[:], p_zp[:])

            p_ap = ps.tile([CLUSTERS, GPD, CLUSTERS], F32, tag="ps")
            for g in range(GPD):
                for c, cn in ((0, C0), (1, C1)):
                    nc.tensor.matmul(p_ap[:, g, :], s_S[c][0:cn, g, :],
                                     s_AS[c][0:cn, g, :], start=(c == 0), stop=(c == 1))
            s_Ap = act.tile([CLUSTERS, GPD, CLUSTERS], F32, tag="apool")
            nc.scalar.copy(s_Ap[:], p_ap[:])

            # ---- pooled normalization ---------------------------------------
            degp = act.tile([CLUSTERS, GPD], F32, tag="degp")
            nc.vector.reduce_sum(out=degp[:], in_=s_Ap[:], axis=mybir.AxisListType.X)
            nc.vector.tensor_scalar_add(degp[:], degp[:], 1.0)
            recp = act.tile([CLUSTERS, GPD], F32, tag="recp")
            nc.vector.reciprocal(recp[:], degp[:])
            dp = act.tile([CLUSTERS, GPD], F32, tag="dp")
            nc.scalar.sqrt(dp[:], recp[:])

            dpTd = dram.tile([GPD * CLUSTERS], F32, tag="dpTd")
            nc.gpsimd.dma_start(out=dpTd[:].rearrange("(g j) -> j g", g=GPD), in_=dp[:])
            s_dpT = act.tile([CLUSTERS, GPD, CLUSTERS], F32, tag="dpT")
            dp_src = dpTd[:].rearrange("(g j) -> g j", g=GPD)[None, :, :]
            nc.gpsimd.dma_start(out=s_dpT[:],
                              in_=dp_src.broadcast_to((CLUSTERS, GPD, CLUSTERS)))

            # Anp = dp_row * (Ap + I) * dp_col, materialized fully (tiny).
            ahp = act.tile([CLUSTERS, GPD, CLUSTERS], F32, tag="ahp")
            idb = s_id25[:][:, None, :].broadcast_to((CLUSTERS, GPD, CLUSTERS))
            nc.gpsimd.tensor_add(ahp[:], s_Ap[:], idb)
            nc.vector.tensor_mul(ahp[:], ahp[:],
                                 dp[:][:, :, None].broadcast_to((CLUSTERS, GPD, CLUSTERS)))
            anp = act.tile([CLUSTERS, GPD, CLUSTERS], F32, tag="anp")
            nc.vector.tensor_mul(anp[:], ahp[:], s_dpT[:])

            # ---- pooled GCN: H = relu(Anp @ (Zp @ Wp) + bp), fm -------------
            p_zw = ps.tile([CLUSTERS, GPD, HIDDEN], F32, tag="ps")
            for g in range(GPD):
                nc.tensor.matmul(p_zw[:, g, :], s_Zp[:, g, :], s_Wp[:],
                                 start=True, stop=True)
            s_ZW = act.tile([CLUSTERS, GPD, HIDDEN], F32, tag="zw")
            nc.vector.tensor_copy(s_ZW[:], p_zw[:])

            p_h = ps.tile([HIDDEN, GPD, CLUSTERS], F32, tag="ps")
            for g in range(GPD):
                nc.tensor.matmul(p_h[:, g, :], s_ZW[:, g, :], anp[:, g, :],
                                 start=True, stop=True)
            s_H = act.tile([HIDDEN, GPD, CLUSTERS], F32, tag="h")
            nc.scalar.activation(s_H[:], p_h[:], AF.Relu, bias=s_bp[:])

            # ---- readout + classifier ---------------------------------------
            s_G = act.tile([HIDDEN, GPD], F32, tag="g")
            nc.vector.reduce_sum(out=s_G[:], in_=s_H[:], axis=mybir.AxisListType.X)

            p_l = ps.tile([GPD, NUM_CLASSES], F32, tag="ps")
            nc.tensor.matmul(p_l[:], s_G[:], s_Wc[:], start=True, stop=True)
            s_out = act.tile([GPD, NUM_CLASSES], F32, tag="logits")
            nc.vector.tensor_add(s_out[:], p_l[:], s_bc[:])
            nc.gpsimd.dma_start(out=out, in_=s_out[:])

    nc.compile()
    return nc


def make_in_maps(x, a, W1, b1, W2, b2, Wa, ba, Wp, bp, Wc, bc):
    x = np.ascontiguousarray(np.asarray(x, dtype=np.float32))
    a = np.asarray(a, dtype=np.float32)

    # diagonal 150x150 blocks of the batch adjacency
    ab = a.reshape(B_GRAPHS, NPG, B_GRAPHS, NPG)
    blocks = ab[np.arange(B_GRAPHS), :, np.arange(B_GRAPHS), :]  # [64, 150, 150]

    id0 = np.zeros((C0, NPG), np.float32)
    id0[np.arange(C0), np.arange(C0)] = 1.0
    id1 = np.zeros((C1, NPG), np.float32)
    id1[np.arange(C1), C0 + np.arange(C1)] = 1.0

    common = dict(
        id0=id0, id1=id1,
        id25=np.eye(CLUSTERS, dtype=np.float32),
        id64=np.eye(HIDDEN, dtype=np.float32),
        W1=np.asarray(W1, np.float32), W2=np.asarray(W2, np.float32),
        Wa=np.asarray(Wa, np.float32), Wp=np.asarray(Wp, np.float32),
        Wc=np.asarray(Wc, np.float32),
        b1=np.asarray(b1, np.float32).reshape(HIDDEN, 1),
        b2=np.asarray(b2, np.float32).reshape(HIDDEN, 1),
        bp=np.asarray(bp, np.float32).reshape(HIDDEN, 1),
        ba=np.asarray(ba, np.float32).reshape(1, CLUSTERS),
        bc=np.asarray(bc, np.float32).reshape(1, NUM_CLASSES),
    )

    in_maps = []
    for d in range(DEV):
        xd = x[d * GPD * NPG:(d + 1) * GPD * NPG]          # [1200, 128]
        xT = np.ascontiguousarray(xd.T).reshape(N_FEAT, GPD, NPG)
        bd = blocks[d * GPD:(d + 1) * GPD]                  # [8, 150, 150]
        bt = np.ascontiguousarray(bd.transpose(1, 0, 2))    # [150, 8, 150]
        in_maps.append(dict(
            xT=xT,
            a0=np.ascontiguousarray(bt[:C0]),
            a1=np.ascontiguousarray(bt[C0:]),
            **common,
        ))
    return in_maps


def kernel(x, a, seg_ids, num_graphs, W1, b1, W2, b2, Wa, ba, Wp, bp, Wc, bc,
           trace=False):
    if "nc" not in _CACHE:
        _CACHE["nc"] = build_nc()
    nc = _CACHE["nc"]
    in_maps = make_in_maps(x, a, W1, b1, W2, b2, Wa, ba, Wp, bp, Wc, bc)
    res = run_bass_kernel_spmd(nc, in_maps, core_ids=list(range(DEV)), trace=trace)
    logits = np.concatenate([r["out"] for r in res.results], axis=0)
    if trace:
        return logits, res
    return logits
